# revision 52
# baseline (speedup 1.0000x reference)
"""GQA causal-attention prefill kernel for Trainium2 (8 NeuronCores), v4.

Sharding: head-parallel - core c gets query heads 4c..4c+3 and kv head c.

Per-core device algorithm (matmuls bf16, scores transposed [key, q]):
  S^T[key, q] = kT_blk.T @ qT_blk                  (PE, exact causal widths)
  P^T = exp(SCALE * S^T) split across engines:
    - non-diag groups: ACT real exp, or Pool Schraudolph bit-trick
    - diag groups: DVE/Pool scalar_tensor_tensor fused Schraudolph+mask:
        i16 = rint(S*A + Bmask),  Bmask = B_S (live) / B_S-4000 (masked)
      masked entries decode to ~2^-25 in bf16 -> effectively zero, so no
      separate mask multiplies are needed anywhere.
  outT[d, q] += V_blk.T @ P^T_blk                  (PE, PSUM accumulate)
  pair[key, q] = P^T_blk0 + P^T_blk1 ship slots    (DVE adds/copies)
  out-copy PSUM->SBUF bf16                         (ACT)
  DMA out: unnormalized outT (bf16) + per-(h,M) ship tile of pair sums

(h, M) processed M-descending so the kernel ends on the small M=0 chunk
(short tail).  Host: l[q] = sum over shipped pair rows; out = outT / l.
"""

import numpy as np
import ml_dtypes

BF16 = ml_dtypes.bfloat16

SEQ = 2048
NUM_HEADS = 32
NUM_KV_HEADS = 8
D = 128
NCORES = 8
HPC = NUM_HEADS // NCORES
SCALE = float(1.0 / np.sqrt(D))

P = 128
QB = 512
NQB = SEQ // QB
NKB = SEQ // P
NSLOT = 2 * NQB  # max pair slots per (h, M): 2M+2 <= 8

A_S = float(SCALE * 128.0 / np.log(2.0))
B_S = 16255.0
B_MASKED = B_S - 4000.0  # decodes to ~2^-25: effectively zero post-bitcast

_COMPILED = {}


def _groups():
    """Global group order: per head M descending (3,2,1,0), but the small
    M=1 / M=0 chunks are woven into the NEXT head's M=3 stream so PE always
    has independent QK work while their exps drain.  At most 2 chunks are
    ever open simultaneously (accpool bufs=2)."""
    def chunk(h, M):
        npairs = 2 * (M + 1)
        return [(h, M, gp, gp == 0, gp == npairs - 1) for gp in range(npairs)]

    def weave(a, b):
        # a: earlier chunk's remaining groups, b: next chunk's groups.
        # alternate 1:1 starting with a, then append the rest.
        out = []
        ia = ib = 0
        while ia < len(a) or ib < len(b):
            if ia < len(a):
                out.append(a[ia]); ia += 1
            if ib < len(b):
                out.append(b[ib]); ib += 1
        return out

    gs = []
    carry_prev = []
    for h in range(HPC):
        gs += weave(carry_prev, chunk(h, 3))
        if h == HPC - 1:
            gs += chunk(h, 2) + chunk(h, 1) + chunk(h, 0)
        else:
            gs += chunk(h, 2)
            carry_prev = chunk(h, 1) + chunk(h, 0)
    return gs


def _ne_engine(M, gp, t):
    # non-diagonal exp engine per half-group (2*gp+t).  GPSIMD cannot read
    # PSUM on TRN2, so exps live on ACT ('A', real exp) / DVE ('V',
    # Schraudolph); Pool instead absorbs the SBUF-side ship work.
    pat = {3: "AAVAAAAVAAVA", 2: "AAAVAAAV", 1: "AAAA"}[M]
    return pat[2 * gp + t]


def _padd_engine(M, gp):
    # pair-add engine for non-diag ship slots: ~20/48 on Pool
    return "P" if (M == 3 and gp in (0, 3)) or (M == 2 and gp in (0, 2))         or (M == 1 and gp == 0) else "V"


def _build(num_devices=NCORES, reps=1):
    import concourse.mybir as mybir
    import concourse.tile as tile
    from concourse import bacc

    f32 = mybir.dt.float32
    bf16 = mybir.dt.bfloat16
    i16 = mybir.dt.int16
    Exp = mybir.ActivationFunctionType.Exp
    Copy = mybir.ActivationFunctionType.Copy
    MULT = mybir.AluOpType.mult
    ADD = mybir.AluOpType.add

    nc = bacc.Bacc(
        "TRN2", target_bir_lowering=False, debug=False, num_devices=num_devices
    )

    qT_d = nc.dram_tensor("qT", [HPC, P, SEQ], bf16, kind="ExternalInput")
    kT_d = nc.dram_tensor("kT", [P, SEQ], bf16, kind="ExternalInput")
    v_d = nc.dram_tensor("v", [P, SEQ], bf16, kind="ExternalInput")
    # h0: host-packed [kT blocks 0-1 | q head0 M3] so ONE critical DMA
    # unblocks the first QK matmuls (each DMA chain costs ~2.2us latency)
    h0_d = nc.dram_tensor("h0", [P, 2 * P + QB], bf16, kind="ExternalInput")
    bm_d = nc.dram_tensor("bmask", [P, 2 * QB + 384], i16, kind="ExternalInput")
    outT_d = nc.dram_tensor("outT", [HPC, NQB, P, QB], bf16, kind="ExternalOutput")
    # ship layout: [h, M, partition(key), slot, q] so SBUF [p, slot, q] maps 1:1
    acc_d = nc.dram_tensor("acc", [HPC, NQB, P, NSLOT, QB], bf16,
                           kind="ExternalOutput")

    with tile.TileContext(nc) as tc:
        with (
            tc.tile_pool(name="const", bufs=1) as cpool,
            tc.tile_pool(name="pt", bufs=10, space="SBUF") as ptpool,
            tc.tile_pool(name="ship", bufs=2) as shpool,
            tc.tile_pool(name="ob", bufs=3) as obpool,
            tc.tile_pool(name="st", bufs=6, space="PSUM") as stpool,
            tc.tile_pool(name="acc", bufs=2, space="PSUM") as accpool,
        ):
            # h0 carries kT blocks 0-1 + q head0 M3; rest of kT/v split in
            # need-ordered chunks so early blocks land fast
            h0_sb = cpool.tile([P, 2 * P + QB], bf16, tag="h0")
            kT0b = cpool.tile([P, 2 * P], bf16, tag="kT0b")
            kT1 = cpool.tile([P, QB], bf16, tag="kT1")
            kT2a = cpool.tile([P, QB], bf16, tag="kT2a")
            kT2b = cpool.tile([P, QB], bf16, tag="kT2b")
            v_sb = [
                cpool.tile([P, QB], bf16, tag=f"v{i}", name=f"v_sb{i}")
                for i in range(4)
            ]
            qB0 = cpool.tile([P, 3 * QB], bf16, tag="qB0")
            q_sb = [
                cpool.tile([P, SEQ], bf16, tag=f"q{h}", name=f"q_sb{h}")
                for h in range(1, HPC)
            ]
            bm_sb = cpool.tile([P, 2 * QB + 384], i16, tag="bm")
            warm_sb = cpool.tile([P, 1], f32, tag="warm")

            # input DMAs: strictly need-ordered on SP (the shared HWDGE +
            # DMA-engine devices serialize them); qB0 rides the ACT queue.
            nc.sync.dma_start(h0_sb[:], h0_d.ap())
            nc.vector.memset(warm_sb[:], 0.0)
            nc.scalar.activation(warm_sb[:], warm_sb[:], Exp, scale=SCALE)
            nc.sync.dma_start(kT0b[:], kT_d.ap()[:, 2 * P : QB])
            nc.sync.dma_start(kT1[:], kT_d.ap()[:, QB : 2 * QB])
            nc.sync.dma_start(kT2a[:], kT_d.ap()[:, 2 * QB : 3 * QB])
            nc.sync.dma_start(v_sb[0][:], v_d.ap()[:, 0:QB])
            nc.sync.dma_start(kT2b[:], kT_d.ap()[:, 3 * QB : 4 * QB])
            nc.sync.dma_start(bm_sb[:], bm_d.ap())
            nc.sync.dma_start(v_sb[1][:], v_d.ap()[:, QB : 2 * QB])
            nc.sync.dma_start(v_sb[2][:], v_d.ap()[:, 2 * QB : 3 * QB])
            nc.sync.dma_start(qB0[:], qT_d.ap()[0][:, 0 : 3 * QB])
            nc.sync.dma_start(v_sb[3][:], v_d.ap()[:, 3 * QB : 4 * QB])
            for h in range(1, HPC):
                nc.sync.dma_start(q_sb[h - 1][:], qT_d.ap()[h])

            def kT_blk(j):
                if j < 2:
                    return h0_sb[:, j * P : (j + 1) * P]
                if j < 4:
                    return kT0b[:, (j - 2) * P : (j - 1) * P]
                t = [None, None, None, None, kT1, kT1, kT1, kT1,
                     kT2a, kT2a, kT2a, kT2a, kT2b, kT2b, kT2b, kT2b][j]
                return t[:, (j % 4) * P : (j % 4 + 1) * P]

            def v_blk(j):
                return v_sb[j // 4][:, (j % 4) * P : (j % 4 + 1) * P]

            def q_tile(h, M):
                if h == 0:
                    if M == 3:
                        return h0_sb[:, 2 * P : 2 * P + QB]
                    return qB0[:, M * QB : (M + 1) * QB]
                return q_sb[h - 1][:, M * QB : (M + 1) * QB]

            groups = _groups()
            last_idx = len(groups) - 1

            state = {}

            def produce(idx):
                h, M, gp, _, _ = groups[idx]
                rep = state.get("rep", 0)
                # one single-bank PSUM tile per key block (half-group) so PE
                # can run 6 blocks ahead of exp completion
                pt = ptpool.tile([P, 2 * QB], bf16, tag="pt", name=f"pt{rep}_{idx}")
                if gp == 2 * M + 1:
                    # diag B: both key blocks' live cols pack into ONE tile
                    # ([0:256) = blk u=2 cols [2P:QB), [256:384) = blk u=3
                    # cols [3P:QB)) -> single fused exp+mask op on DVE
                    st = stpool.tile(
                        [P, QB], f32, tag="st", name=f"st{rep}_{idx}_b"
                    )
                    nc.tensor.matmul(
                        st[:, 0 : 2 * P],
                        lhsT=kT_blk(4 * M + 2),
                        rhs=q_tile(h, M)[:, 2 * P : QB],
                        start=True, stop=True,
                    )
                    nc.tensor.matmul(
                        st[:, 2 * P : 3 * P],
                        lhsT=kT_blk(4 * M + 3),
                        rhs=q_tile(h, M)[:, 3 * P : QB],
                        start=True, stop=True,
                    )
                    nc.vector.scalar_tensor_tensor(
                        pt[:, 0 : 3 * P].bitcast(i16), st[:, 0 : 3 * P],
                        A_S, bm_sb[:, 2 * QB : 2 * QB + 3 * P],
                        op0=MULT, op1=ADD,
                    )
                    state[idx] = pt
                    return
                for t in range(2):
                    j = 2 * gp + t
                    u = j - 4 * M
                    lo = u * P if u > 0 else 0
                    st = stpool.tile(
                        [P, QB], f32, tag="st", name=f"st{rep}_{idx}_{t}"
                    )
                    nc.tensor.matmul(
                        st[:, lo:QB],
                        lhsT=kT_blk(j),
                        rhs=q_tile(h, M)[:, lo:QB],
                        start=True,
                        stop=True,
                    )
                    po = t * QB  # pt column offset for this half
                    if gp == 2 * M:
                        # diag: fused Schraudolph exp + causal mask on DVE
                        nc.vector.scalar_tensor_tensor(
                            pt[:, po : po + QB].bitcast(i16), st[:], A_S,
                            bm_sb[:, t * QB : (t + 1) * QB],
                            op0=MULT, op1=ADD,
                        )
                    elif gp == 2 * M + 1:
                        # diag B: live cols [2P:QB); same bmask pattern shifted
                        nc.vector.scalar_tensor_tensor(
                            pt[:, po + 2 * P : po + QB].bitcast(i16),
                            st[:, 2 * P :],
                            A_S,
                            bm_sb[:, t * QB : t * QB + 2 * P],
                            op0=MULT, op1=ADD,
                        )
                    else:
                        if _ne_engine(M, gp, t) == "A":
                            nc.scalar.activation(
                                pt[:, po : po + QB], st[:], Exp, scale=SCALE
                            )
                        else:
                            nc.vector.tensor_scalar(
                                pt[:, po : po + QB].bitcast(i16), st[:],
                                A_S, B_S, op0=MULT, op1=ADD,
                            )
                state[idx] = pt

            def consume(idx):
                h, M, gp, first, last = groups[idx]
                rep = state.get("rep", 0)
                pt = state.pop(idx)
                if first:
                    state["out_ps", h, M] = accpool.tile(
                        [P, QB], f32, tag="out", name=f"out{rep}_{h}_{M}"
                    )
                    state["ship", h, M] = shpool.tile(
                        [P, 2 * M + 2, QB], bf16, tag=f"ship{M}", name=f"sh{rep}_{h}_{M}"
                    )
                out_ps = state["out_ps", h, M]
                ship = state["ship", h, M]
                if gp == 2 * M + 1:
                    nc.tensor.matmul(
                        out_ps[:, 2 * P : QB],
                        lhsT=v_blk(4 * M + 2),
                        rhs=pt[:, 0 : 2 * P],
                        start=False, stop=False,
                    )
                    nc.tensor.matmul(
                        out_ps[:, 3 * P : QB],
                        lhsT=v_blk(4 * M + 3),
                        rhs=pt[:, 2 * P : 3 * P],
                        start=False, stop=last,
                    )
                else:
                    for t in range(2):
                        j = 2 * gp + t
                        u = j - 4 * M
                        lo = u * P if u > 0 else 0
                        nc.tensor.matmul(
                            out_ps[:, lo:QB],
                            lhsT=v_blk(j),
                            rhs=pt[:, t * QB + lo : (t + 1) * QB],
                            start=(first and t == 0),
                            stop=(last and t == 1),
                        )
                tail_split = False
                copy_split = False
                tail_chunk = h == HPC - 1 and M <= 1
                if gp == 2 * M:
                    # pair A -> slot 2M: [0,P) only block0; add the rest.
                    # Pool normally; DVE for the tail chunks (idle then,
                    # and the final ship DMAs wait on these)
                    if tail_chunk:
                        nc.vector.tensor_copy(ship[:, gp, 0:P], pt[:, 0:P])
                        nc.vector.tensor_add(
                            ship[:, gp, P:QB], pt[:, P:QB],
                            pt[:, QB + P : 2 * QB],
                        )
                    else:
                        nc.gpsimd.tensor_scalar(
                            ship[:, gp, 0:P], pt[:, 0:P], 1.0, 0.0,
                            op0=MULT, op1=ADD,
                        )
                        nc.gpsimd.tensor_tensor(
                            ship[:, gp, P:QB], pt[:, P:QB],
                            pt[:, QB + P : 2 * QB], op=ADD,
                        )
                    if tail_split:
                        nc.sync.dma_start(
                            acc_d.ap()[h][M][:, 2 * M : 2 * M + 1, :],
                            ship[:, 2 * M : 2 * M + 1, :],
                        )
                    if copy_split:
                        # cols [0, 2P) of out_ps are final after diag-A PVs:
                        # copy+ship them while diag-B still accumulates hi cols
                        osb = obpool.tile(
                            [P, QB], bf16, tag="ob", name=f"ob{rep}_{h}{M}"
                        )
                        state["osb", h, M] = osb
                        nc.scalar.activation(
                            osb[:, 0 : 2 * P], out_ps[:, 0 : 2 * P], Copy
                        )
                        nc.scalar.dma_start(
                            outT_d.ap()[h][M][:, 0 : 2 * P], osb[:, 0 : 2 * P]
                        )
                elif gp == 2 * M + 1:
                    # pair B -> slot 2M+1, live [2P, QB); host ignores [0, 2P)
                    if tail_chunk:
                        nc.vector.tensor_copy(
                            ship[:, gp, 2 * P : 3 * P], pt[:, 0:P]
                        )
                        nc.vector.tensor_add(
                            ship[:, gp, 3 * P : QB],
                            pt[:, P : 2 * P], pt[:, 2 * P : 3 * P],
                        )
                    else:
                        nc.gpsimd.tensor_scalar(
                            ship[:, gp, 2 * P : 3 * P], pt[:, 0:P],
                            1.0, 0.0, op0=MULT, op1=ADD,
                        )
                        nc.gpsimd.tensor_tensor(
                            ship[:, gp, 3 * P : QB],
                            pt[:, P : 2 * P],
                            pt[:, 2 * P : 3 * P], op=ADD,
                        )
                    nc.sync.dma_start(
                        acc_d.ap()[h][M][:, 2 * M : 2 * M + 2, :],
                        ship[:, 2 * M : 2 * M + 2, :],
                    )
                else:
                    if _padd_engine(M, gp) == "V":
                        nc.vector.tensor_add(
                            ship[:, gp], pt[:, 0:QB], pt[:, QB : 2 * QB]
                        )
                    else:
                        nc.gpsimd.tensor_tensor(
                            ship[:, gp], pt[:, 0:QB], pt[:, QB : 2 * QB],
                            op=ADD,
                        )
                    if gp == 2 * M - 1:
                        # bulk ship: all non-diag slots [0, 2M) complete
                        nc.sync.dma_start(
                            acc_d.ap()[h][M][:, 0 : 2 * M, :],
                            ship[:, 0 : 2 * M, :],
                        )

                if last:
                    osb = obpool.tile(
                        [P, QB], bf16, tag="ob", name=f"ob{rep}_{h}{M}"
                    )
                    if h == HPC - 1 and M == 0:
                        # last chunk: halve the post-PE copy latency by
                        # splitting it across ACT + DVE (both idle then)
                        nc.scalar.activation(
                            osb[:, 2 * P :], out_ps[:, 2 * P :], Copy
                        )
                        nc.vector.tensor_copy(
                            osb[:, 0 : 2 * P], out_ps[:, 0 : 2 * P]
                        )
                    else:
                        nc.scalar.activation(osb[:], out_ps[:], Copy)
                    # last head's outT DMAs dispatch on the (then-idle)
                    # ACT queue so they don't head-of-line block the
                    # tail ship DMAs on SP
                    if h == HPC - 1:
                        nc.scalar.dma_start(outT_d.ap()[h][M], osb[:])
                    else:
                        nc.sync.dma_start(outT_d.ap()[h][M], osb[:])
                    del state["out_ps", h, M]
                    del state["ship", h, M]

            LOOKAHEAD = 8
            for rep in range(reps):
                state["rep"] = rep
                for i in range(min(LOOKAHEAD, len(groups))):
                    produce(i)
                for i in range(len(groups)):
                    if i + LOOKAHEAD < len(groups):
                        produce(i + LOOKAHEAD)
                    consume(i)

    nc.compile()
    return nc


def _host_bmask():
    p = np.arange(P)[:, None, None]
    t = np.arange(2)[None, :, None]
    c = np.arange(QB)[None, None, :]
    bm = np.where(c >= 128 * t + p, B_S, B_MASKED).astype(np.int16)
    bm = bm.reshape(P, 2 * QB)
    # packed diag-B region: [tri over 256 cols | tri over 128 cols]
    p2 = np.arange(P)[:, None]
    c256 = np.arange(2 * P)[None, :]
    c128 = np.arange(P)[None, :]
    tri256 = np.where(c256 >= p2, B_S, B_MASKED).astype(np.int16)
    tri128 = np.where(c128 >= p2, B_S, B_MASKED).astype(np.int16)
    return np.concatenate([bm, tri256, tri128], axis=1)


def _pack_inputs(q, k, v):
    """Per-core input dict list (shared by kernel() and test harnesses)."""
    bm = _host_bmask()
    in_maps = []
    for c in range(NCORES):
        qT_c = np.ascontiguousarray(
            q[:, HPC * c : HPC * (c + 1), :].transpose(1, 2, 0)
        ).astype(BF16)
        kT_c = np.ascontiguousarray(k[:, c, :].T).astype(BF16)
        v_c = np.ascontiguousarray(
            v[:, c, :].reshape(NKB, P, D).transpose(1, 0, 2).reshape(P, SEQ)
        ).astype(BF16)
        h0_c = np.ascontiguousarray(
            np.concatenate([kT_c[:, 0 : 2 * P], qT_c[0, :, 3 * QB :]], axis=1)
        )
        in_maps.append(
            {"qT": qT_c, "kT": kT_c, "v": v_c, "bmask": bm, "h0": h0_c}
        )
    return in_maps


def kernel(q, k, v, k_cache=None, v_cache=None, slot_mapping=None, **_):
    from concourse.bass_utils import run_bass_kernel_spmd

    if "nc" not in _COMPILED:
        _COMPILED["nc"] = _build()
    nc = _COMPILED["nc"]

    q = np.asarray(q, dtype=np.float32)
    k = np.asarray(k, dtype=np.float32)
    v = np.asarray(v, dtype=np.float32)

    in_maps = _pack_inputs(q, k, v)
    res = run_bass_kernel_spmd(nc, in_maps, list(range(NCORES)))

    out = np.empty((SEQ, NUM_HEADS, D), np.float32)
    for c in range(NCORES):
        oT = res.results[c]["outT"].astype(np.float32)   # [HPC, NQB, d, q]
        ac = res.results[c]["acc"]                        # [HPC, NQB, p, slot, q]
        for h in range(HPC):
            for M in range(NQB):
                a = ac[h, M].astype(np.float32)  # [128, NSLOT, 512]
                l = a[:, 0 : 2 * M + 1, :].sum(axis=(0, 1))  # pairs + pair A
                l[2 * P :] += a[:, 2 * M + 1, 2 * P :].sum(axis=0)  # pair B
                out[M * QB : (M + 1) * QB, HPC * c + h, :] = (oT[h, M] / l).T
    return out


# revision 53
# speedup vs baseline: 1.0095x; 1.0095x over previous
"""GQA causal-attention prefill kernel for Trainium2 (8 NeuronCores), v4.

Sharding: head-parallel - core c gets query heads 4c..4c+3 and kv head c.

Per-core device algorithm (matmuls bf16, scores transposed [key, q]):
  S^T[key, q] = kT_blk.T @ qT_blk                  (PE, exact causal widths)
  P^T = exp(SCALE * S^T) split across engines:
    - non-diag groups: ACT real exp, or Pool Schraudolph bit-trick
    - diag groups: DVE/Pool scalar_tensor_tensor fused Schraudolph+mask:
        i16 = rint(S*A + Bmask),  Bmask = B_S (live) / B_S-4000 (masked)
      masked entries decode to ~2^-25 in bf16 -> effectively zero, so no
      separate mask multiplies are needed anywhere.
  outT[d, q] += V_blk.T @ P^T_blk                  (PE, PSUM accumulate)
  pair[key, q] = P^T_blk0 + P^T_blk1 ship slots    (DVE adds/copies)
  out-copy PSUM->SBUF bf16                         (ACT)
  DMA out: unnormalized outT (bf16) + per-(h,M) ship tile of pair sums

(h, M) processed M-descending so the kernel ends on the small M=0 chunk
(short tail).  Host: l[q] = sum over shipped pair rows; out = outT / l.
"""

import numpy as np
import ml_dtypes

BF16 = ml_dtypes.bfloat16

SEQ = 2048
NUM_HEADS = 32
NUM_KV_HEADS = 8
D = 128
NCORES = 8
HPC = NUM_HEADS // NCORES
SCALE = float(1.0 / np.sqrt(D))

P = 128
QB = 512
NQB = SEQ // QB
NKB = SEQ // P
NSLOT = 2 * NQB  # max pair slots per (h, M): 2M+2 <= 8

A_S = float(SCALE * 128.0 / np.log(2.0))
B_S = 16255.0
B_MASKED = B_S - 4000.0  # decodes to ~2^-25: effectively zero post-bitcast

_COMPILED = {}


def _groups():
    """Global group order: per head M descending (3,2,1,0), but the small
    M=1 / M=0 chunks are woven into the NEXT head's M=3 stream so PE always
    has independent QK work while their exps drain.  At most 2 chunks are
    ever open simultaneously (accpool bufs=2)."""
    def chunk(h, M):
        npairs = 2 * (M + 1)
        return [(h, M, gp, gp == 0, gp == npairs - 1) for gp in range(npairs)]

    def weave(a, b):
        # a: earlier chunk's remaining groups, b: next chunk's groups.
        # alternate 1:1 starting with a, then append the rest.
        out = []
        ia = ib = 0
        while ia < len(a) or ib < len(b):
            if ia < len(a):
                out.append(a[ia]); ia += 1
            if ib < len(b):
                out.append(b[ib]); ib += 1
        return out

    gs = []
    carry_prev = []
    for h in range(HPC):
        gs += weave(carry_prev, chunk(h, 3))
        if h == HPC - 1:
            gs += chunk(h, 2) + chunk(h, 1) + chunk(h, 0)
        else:
            gs += chunk(h, 2)
            carry_prev = chunk(h, 1) + chunk(h, 0)
    return gs


def _ne_engine(M, gp, t):
    # non-diagonal exp engine per half-group (2*gp+t).  GPSIMD cannot read
    # PSUM on TRN2, so exps live on ACT ('A', real exp) / DVE ('V',
    # Schraudolph); Pool instead absorbs the SBUF-side ship work.
    pat = {3: "AAVAAAAVAAVA", 2: "AAAVAAAV", 1: "AAAA"}[M]
    return pat[2 * gp + t]


def _padd_engine(M, gp):
    # pair-add engine for non-diag ship slots: ~20/48 on Pool
    return "P" if (M == 3 and gp in (0, 3)) or (M == 2 and gp in (0, 2))         or (M == 1 and gp == 0) else "V"


def _build(num_devices=NCORES, reps=1):
    import concourse.mybir as mybir
    import concourse.tile as tile
    from concourse import bacc

    f32 = mybir.dt.float32
    bf16 = mybir.dt.bfloat16
    i16 = mybir.dt.int16
    Exp = mybir.ActivationFunctionType.Exp
    Copy = mybir.ActivationFunctionType.Copy
    MULT = mybir.AluOpType.mult
    ADD = mybir.AluOpType.add

    nc = bacc.Bacc(
        "TRN2", target_bir_lowering=False, debug=False, num_devices=num_devices
    )

    qT_d = nc.dram_tensor("qT", [HPC, P, SEQ], bf16, kind="ExternalInput")
    kT_d = nc.dram_tensor("kT", [P, SEQ], bf16, kind="ExternalInput")
    v_d = nc.dram_tensor("v", [P, SEQ], bf16, kind="ExternalInput")
    # h0: host-packed [kT blocks 0-1 | q head0 M3] so ONE critical DMA
    # unblocks the first QK matmuls (each DMA chain costs ~2.2us latency)
    h0_d = nc.dram_tensor("h0", [P, 2 * P + QB], bf16, kind="ExternalInput")
    bm_d = nc.dram_tensor("bmask", [P, 2 * QB + 384], i16, kind="ExternalInput")
    outT_d = nc.dram_tensor("outT", [HPC, NQB, P, QB], bf16, kind="ExternalOutput")
    # ship layout: [h, M, partition(key), slot, q] so SBUF [p, slot, q] maps 1:1
    acc_d = nc.dram_tensor("acc", [HPC, NQB, P, NSLOT, QB], bf16,
                           kind="ExternalOutput")

    with tile.TileContext(nc) as tc:
        with (
            tc.tile_pool(name="const", bufs=1) as cpool,
            tc.tile_pool(name="pt", bufs=10, space="SBUF") as ptpool,
            tc.tile_pool(name="ship", bufs=2) as shpool,
            tc.tile_pool(name="ob", bufs=3) as obpool,
            tc.tile_pool(name="st", bufs=6, space="PSUM") as stpool,
            tc.tile_pool(name="acc", bufs=2, space="PSUM") as accpool,
        ):
            # h0 carries kT blocks 0-1 + q head0 M3; rest of kT/v split in
            # need-ordered chunks so early blocks land fast
            h0_sb = cpool.tile([P, 2 * P + QB], bf16, tag="h0")
            kT0b = cpool.tile([P, 2 * P], bf16, tag="kT0b")
            kT1 = cpool.tile([P, QB], bf16, tag="kT1")
            kT2a = cpool.tile([P, QB], bf16, tag="kT2a")
            kT2b = cpool.tile([P, QB], bf16, tag="kT2b")
            v_sb = [
                cpool.tile([P, QB], bf16, tag=f"v{i}", name=f"v_sb{i}")
                for i in range(4)
            ]
            qB0 = cpool.tile([P, 3 * QB], bf16, tag="qB0")
            q_sb = [
                cpool.tile([P, SEQ], bf16, tag=f"q{h}", name=f"q_sb{h}")
                for h in range(1, HPC)
            ]
            bm_sb = cpool.tile([P, 2 * QB + 384], i16, tag="bm")
            warm_sb = cpool.tile([P, 1], f32, tag="warm")

            # input DMAs: strictly need-ordered on SP (the shared HWDGE +
            # DMA-engine devices serialize them); qB0 rides the ACT queue.
            nc.sync.dma_start(h0_sb[:], h0_d.ap())
            nc.vector.memset(warm_sb[:], 0.0)
            nc.scalar.activation(warm_sb[:], warm_sb[:], Exp, scale=SCALE)
            nc.sync.dma_start(kT0b[:], kT_d.ap()[:, 2 * P : QB])
            nc.sync.dma_start(kT1[:], kT_d.ap()[:, QB : 2 * QB])
            nc.sync.dma_start(kT2a[:], kT_d.ap()[:, 2 * QB : 3 * QB])
            nc.sync.dma_start(v_sb[0][:], v_d.ap()[:, 0:QB])
            nc.sync.dma_start(kT2b[:], kT_d.ap()[:, 3 * QB : 4 * QB])
            nc.sync.dma_start(bm_sb[:], bm_d.ap())
            nc.sync.dma_start(v_sb[1][:], v_d.ap()[:, QB : 2 * QB])
            nc.sync.dma_start(v_sb[2][:], v_d.ap()[:, 2 * QB : 3 * QB])
            nc.sync.dma_start(qB0[:], qT_d.ap()[0][:, 0 : 3 * QB])
            nc.sync.dma_start(v_sb[3][:], v_d.ap()[:, 3 * QB : 4 * QB])
            for h in range(1, HPC):
                nc.sync.dma_start(q_sb[h - 1][:], qT_d.ap()[h])

            def kT_blk(j):
                if j < 2:
                    return h0_sb[:, j * P : (j + 1) * P]
                if j < 4:
                    return kT0b[:, (j - 2) * P : (j - 1) * P]
                t = [None, None, None, None, kT1, kT1, kT1, kT1,
                     kT2a, kT2a, kT2a, kT2a, kT2b, kT2b, kT2b, kT2b][j]
                return t[:, (j % 4) * P : (j % 4 + 1) * P]

            def v_blk(j):
                return v_sb[j // 4][:, (j % 4) * P : (j % 4 + 1) * P]

            def q_tile(h, M):
                if h == 0:
                    if M == 3:
                        return h0_sb[:, 2 * P : 2 * P + QB]
                    return qB0[:, M * QB : (M + 1) * QB]
                return q_sb[h - 1][:, M * QB : (M + 1) * QB]

            groups = _groups()
            last_idx = len(groups) - 1

            state = {}

            def produce(idx):
                h, M, gp, _, _ = groups[idx]
                rep = state.get("rep", 0)
                # one single-bank PSUM tile per key block (half-group) so PE
                # can run 6 blocks ahead of exp completion
                pt = ptpool.tile([P, 2 * QB], bf16, tag="pt", name=f"pt{rep}_{idx}")
                if gp == 2 * M + 1:
                    # diag B: both key blocks' live cols pack into ONE tile
                    # ([0:256) = blk u=2 cols [2P:QB), [256:384) = blk u=3
                    # cols [3P:QB)) -> single fused exp+mask op on DVE
                    st = stpool.tile(
                        [P, QB], f32, tag="st", name=f"st{rep}_{idx}_b"
                    )
                    nc.tensor.matmul(
                        st[:, 0 : 2 * P],
                        lhsT=kT_blk(4 * M + 2),
                        rhs=q_tile(h, M)[:, 2 * P : QB],
                        start=True, stop=True,
                    )
                    nc.tensor.matmul(
                        st[:, 2 * P : 3 * P],
                        lhsT=kT_blk(4 * M + 3),
                        rhs=q_tile(h, M)[:, 3 * P : QB],
                        start=True, stop=True,
                    )
                    nc.vector.scalar_tensor_tensor(
                        pt[:, 0 : 3 * P].bitcast(i16), st[:, 0 : 3 * P],
                        A_S, bm_sb[:, 2 * QB : 2 * QB + 3 * P],
                        op0=MULT, op1=ADD,
                    )
                    state[idx] = pt
                    return
                for t in range(2):
                    j = 2 * gp + t
                    u = j - 4 * M
                    lo = u * P if u > 0 else 0
                    st = stpool.tile(
                        [P, QB], f32, tag="st", name=f"st{rep}_{idx}_{t}"
                    )
                    nc.tensor.matmul(
                        st[:, lo:QB],
                        lhsT=kT_blk(j),
                        rhs=q_tile(h, M)[:, lo:QB],
                        start=True,
                        stop=True,
                    )
                    po = t * QB  # pt column offset for this half
                    if gp == 2 * M:
                        # diag: fused Schraudolph exp + causal mask on DVE
                        nc.vector.scalar_tensor_tensor(
                            pt[:, po : po + QB].bitcast(i16), st[:], A_S,
                            bm_sb[:, t * QB : (t + 1) * QB],
                            op0=MULT, op1=ADD,
                        )
                    elif gp == 2 * M + 1:
                        # diag B: live cols [2P:QB); same bmask pattern shifted
                        nc.vector.scalar_tensor_tensor(
                            pt[:, po + 2 * P : po + QB].bitcast(i16),
                            st[:, 2 * P :],
                            A_S,
                            bm_sb[:, t * QB : t * QB + 2 * P],
                            op0=MULT, op1=ADD,
                        )
                    else:
                        if _ne_engine(M, gp, t) == "A":
                            nc.scalar.activation(
                                pt[:, po : po + QB], st[:], Exp, scale=SCALE
                            )
                        else:
                            nc.vector.tensor_scalar(
                                pt[:, po : po + QB].bitcast(i16), st[:],
                                A_S, B_S, op0=MULT, op1=ADD,
                            )
                state[idx] = pt

            def consume(idx):
                h, M, gp, first, last = groups[idx]
                rep = state.get("rep", 0)
                pt = state.pop(idx)
                if first:
                    state["out_ps", h, M] = accpool.tile(
                        [P, QB], f32, tag="out", name=f"out{rep}_{h}_{M}"
                    )
                    state["ship", h, M] = shpool.tile(
                        [P, 2 * M + 2, QB], bf16, tag=f"ship{M}", name=f"sh{rep}_{h}_{M}"
                    )
                out_ps = state["out_ps", h, M]
                ship = state["ship", h, M]
                if gp == 2 * M + 1:
                    nc.tensor.matmul(
                        out_ps[:, 2 * P : QB],
                        lhsT=v_blk(4 * M + 2),
                        rhs=pt[:, 0 : 2 * P],
                        start=False, stop=False,
                    )
                    nc.tensor.matmul(
                        out_ps[:, 3 * P : QB],
                        lhsT=v_blk(4 * M + 3),
                        rhs=pt[:, 2 * P : 3 * P],
                        start=False, stop=last,
                    )
                else:
                    for t in range(2):
                        j = 2 * gp + t
                        u = j - 4 * M
                        lo = u * P if u > 0 else 0
                        nc.tensor.matmul(
                            out_ps[:, lo:QB],
                            lhsT=v_blk(j),
                            rhs=pt[:, t * QB + lo : (t + 1) * QB],
                            start=(first and t == 0),
                            stop=(last and t == 1),
                        )
                tail_split = False
                copy_split = False
                tail_chunk = h == HPC - 1 and M <= 1
                if gp == 2 * M:
                    # pair A -> slot 2M: [0,P) only block0; add the rest.
                    # Pool normally; DVE for the tail chunks (idle then,
                    # and the final ship DMAs wait on these)
                    if tail_chunk:
                        nc.vector.tensor_copy(ship[:, gp, 0:P], pt[:, 0:P])
                        nc.vector.tensor_add(
                            ship[:, gp, P:QB], pt[:, P:QB],
                            pt[:, QB + P : 2 * QB],
                        )
                    else:
                        nc.gpsimd.tensor_scalar(
                            ship[:, gp, 0:P], pt[:, 0:P], 1.0, 0.0,
                            op0=MULT, op1=ADD,
                        )
                        nc.gpsimd.tensor_tensor(
                            ship[:, gp, P:QB], pt[:, P:QB],
                            pt[:, QB + P : 2 * QB], op=ADD,
                        )
                    if tail_split:
                        nc.sync.dma_start(
                            acc_d.ap()[h][M][:, 2 * M : 2 * M + 1, :],
                            ship[:, 2 * M : 2 * M + 1, :],
                        )
                    if copy_split:
                        # cols [0, 2P) of out_ps are final after diag-A PVs:
                        # copy+ship them while diag-B still accumulates hi cols
                        osb = obpool.tile(
                            [P, QB], bf16, tag="ob", name=f"ob{rep}_{h}{M}"
                        )
                        state["osb", h, M] = osb
                        nc.scalar.activation(
                            osb[:, 0 : 2 * P], out_ps[:, 0 : 2 * P], Copy
                        )
                        nc.scalar.dma_start(
                            outT_d.ap()[h][M][:, 0 : 2 * P], osb[:, 0 : 2 * P]
                        )
                elif gp == 2 * M + 1:
                    # pair B -> slot 2M+1, live [2P, QB); host ignores [0, 2P)
                    if tail_chunk:
                        nc.vector.tensor_copy(
                            ship[:, gp, 2 * P : 3 * P], pt[:, 0:P]
                        )
                        nc.vector.tensor_add(
                            ship[:, gp, 3 * P : QB],
                            pt[:, P : 2 * P], pt[:, 2 * P : 3 * P],
                        )
                    else:
                        nc.gpsimd.tensor_scalar(
                            ship[:, gp, 2 * P : 3 * P], pt[:, 0:P],
                            1.0, 0.0, op0=MULT, op1=ADD,
                        )
                        nc.gpsimd.tensor_tensor(
                            ship[:, gp, 3 * P : QB],
                            pt[:, P : 2 * P],
                            pt[:, 2 * P : 3 * P], op=ADD,
                        )
                    nc.sync.dma_start(
                        acc_d.ap()[h][M][:, 2 * M : 2 * M + 2, :],
                        ship[:, 2 * M : 2 * M + 2, :],
                    )
                else:
                    if _padd_engine(M, gp) == "V":
                        nc.vector.tensor_add(
                            ship[:, gp], pt[:, 0:QB], pt[:, QB : 2 * QB]
                        )
                    else:
                        nc.gpsimd.tensor_tensor(
                            ship[:, gp], pt[:, 0:QB], pt[:, QB : 2 * QB],
                            op=ADD,
                        )
                    if gp == 2 * M - 1:
                        # bulk ship: all non-diag slots [0, 2M) complete
                        nc.sync.dma_start(
                            acc_d.ap()[h][M][:, 0 : 2 * M, :],
                            ship[:, 0 : 2 * M, :],
                        )

                if last:
                    osb = obpool.tile(
                        [P, QB], bf16, tag="ob", name=f"ob{rep}_{h}{M}"
                    )
                    if h == HPC - 1 and M <= 1:
                        # tail chunks: halve the post-PE copy latency by
                        # splitting it across ACT + DVE (both idle then)
                        nc.scalar.activation(
                            osb[:, 2 * P :], out_ps[:, 2 * P :], Copy
                        )
                        nc.vector.tensor_copy(
                            osb[:, 0 : 2 * P], out_ps[:, 0 : 2 * P]
                        )
                    else:
                        nc.scalar.activation(osb[:], out_ps[:], Copy)
                    # last head's outT DMAs dispatch on the (then-idle)
                    # ACT queue so they don't head-of-line block the
                    # tail ship DMAs on SP
                    if h == HPC - 1:
                        nc.scalar.dma_start(outT_d.ap()[h][M], osb[:])
                    else:
                        nc.sync.dma_start(outT_d.ap()[h][M], osb[:])
                    del state["out_ps", h, M]
                    del state["ship", h, M]

            LOOKAHEAD = 8
            for rep in range(reps):
                state["rep"] = rep
                for i in range(min(LOOKAHEAD, len(groups))):
                    produce(i)
                for i in range(len(groups)):
                    if i + LOOKAHEAD < len(groups):
                        produce(i + LOOKAHEAD)
                    consume(i)

    nc.compile()
    return nc


def _host_bmask():
    p = np.arange(P)[:, None, None]
    t = np.arange(2)[None, :, None]
    c = np.arange(QB)[None, None, :]
    bm = np.where(c >= 128 * t + p, B_S, B_MASKED).astype(np.int16)
    bm = bm.reshape(P, 2 * QB)
    # packed diag-B region: [tri over 256 cols | tri over 128 cols]
    p2 = np.arange(P)[:, None]
    c256 = np.arange(2 * P)[None, :]
    c128 = np.arange(P)[None, :]
    tri256 = np.where(c256 >= p2, B_S, B_MASKED).astype(np.int16)
    tri128 = np.where(c128 >= p2, B_S, B_MASKED).astype(np.int16)
    return np.concatenate([bm, tri256, tri128], axis=1)


def _pack_inputs(q, k, v):
    """Per-core input dict list (shared by kernel() and test harnesses)."""
    bm = _host_bmask()
    in_maps = []
    for c in range(NCORES):
        qT_c = np.ascontiguousarray(
            q[:, HPC * c : HPC * (c + 1), :].transpose(1, 2, 0)
        ).astype(BF16)
        kT_c = np.ascontiguousarray(k[:, c, :].T).astype(BF16)
        v_c = np.ascontiguousarray(
            v[:, c, :].reshape(NKB, P, D).transpose(1, 0, 2).reshape(P, SEQ)
        ).astype(BF16)
        h0_c = np.ascontiguousarray(
            np.concatenate([kT_c[:, 0 : 2 * P], qT_c[0, :, 3 * QB :]], axis=1)
        )
        in_maps.append(
            {"qT": qT_c, "kT": kT_c, "v": v_c, "bmask": bm, "h0": h0_c}
        )
    return in_maps


def kernel(q, k, v, k_cache=None, v_cache=None, slot_mapping=None, **_):
    from concourse.bass_utils import run_bass_kernel_spmd

    if "nc" not in _COMPILED:
        _COMPILED["nc"] = _build()
    nc = _COMPILED["nc"]

    q = np.asarray(q, dtype=np.float32)
    k = np.asarray(k, dtype=np.float32)
    v = np.asarray(v, dtype=np.float32)

    in_maps = _pack_inputs(q, k, v)
    res = run_bass_kernel_spmd(nc, in_maps, list(range(NCORES)))

    out = np.empty((SEQ, NUM_HEADS, D), np.float32)
    for c in range(NCORES):
        oT = res.results[c]["outT"].astype(np.float32)   # [HPC, NQB, d, q]
        ac = res.results[c]["acc"]                        # [HPC, NQB, p, slot, q]
        for h in range(HPC):
            for M in range(NQB):
                a = ac[h, M].astype(np.float32)  # [128, NSLOT, 512]
                l = a[:, 0 : 2 * M + 1, :].sum(axis=(0, 1))  # pairs + pair A
                l[2 * P :] += a[:, 2 * M + 1, 2 * P :].sum(axis=0)  # pair B
                out[M * QB : (M + 1) * QB, HPC * c + h, :] = (oT[h, M] / l).T
    return out
